# revision 37
# baseline (speedup 1.0000x reference)
"""GCN 2-layer message-passing kernel for Trainium2 (8 NeuronCores, Bass/Tile).

Strategy (graph/data parallel), v2:
  - Nodes partitioned into 8 contiguous ranges (6250 per core, padded 6272).
  - Host does INTEGER/index prep only. All FP math runs on device.
  - KEY CHANGE vs v1: gather and the dense transform commute
    (sum_e sel_e * (xW) = (sum_e sel_e * x) W), so the gather tables hold
    RAW dinv-scaled features (x~ = dinv * x for layer 1, x1~ = dinv * x1
    for layer 2) instead of the transformed g = dinv * (x @ W). The @W
    moves into a tiny per-block epilogue (transpose + 3 chained matmuls).
    This deletes the 392-matmul/392-ACT-copy per-core table build and the
    ~90us gather hole it caused; layer-1 gathers start ~40us in, right
    after a single DVE broadcast-multiply builds the x~ table.
  - Per layer, per chunk of 5 destination blocks: one dma_gather per class
    (split across 3 SWDGE queues) fetches all message rows; per 128-edge
    tile a 0/1 selection matrix (DVE is_equal vs iota) scatter-reduces raw
    messages into the block's PSUM accumulator via one PE matmul;
    self-loops via an identity matmul on the own-block rows (DMA'd from
    the table); dst-degree scaling via ACT per-partition scale; then the
    epilogue transform: PE transpose -> SBUF -> (agg @ W + x @ resw +
    bias) chained into a second PSUM bank -> relu.
  - Layer-2 x1~ rows are produced inside the layer-1 block loop and
    AllGathered class-by-class as soon as ready (class split keeps tables
    under the int16 index limit).
  - A tiny warm-up dma_gather at t=0 absorbs the Q7 ucode first-call cost.

kernel(**inputs) takes FULL inputs and returns the FULL [50000, 128]
float32 output.
"""
import sys
from contextlib import ExitStack

import numpy as np

if '/opt/trn_rl_repo' not in sys.path:
    sys.path.insert(0, '/opt/trn_rl_repo')

import ml_dtypes

from concourse import bacc, mybir, tile
from concourse.bass_utils import run_bass_kernel_spmd
from concourse.vector_clock import ScopedClock


def _patched_drain_and_barrier(self, tick_clock, wait_clock):
    """Split the kernel-tail drain's sem waits across single-wait drains:
    walrus's NO_STRUCT codegen rejects >1 sync wait on InstDrain."""
    drain_inst = self.nc.sync.drain()
    wait_clock.add_sem_waits(drain_inst.ins,
                             ScopedClock({None: tick_clock.global_clock}))
    si = drain_inst.ins.sync_info
    if si is not None and si.on_wait is not None and len(si.on_wait) > 1:
        waits = list(si.on_wait)
        del si.on_wait[1:]
        for w in waits[1:]:
            d2 = self.nc.sync.drain()
            si2 = d2.ins.sync_info
            if si2 is None:
                d2.ins.sync_info = mybir.SyncInfo(on_wait=[w], on_update=[])
            else:
                si2.on_wait.append(w)
    self.nc.all_engine_barrier()
    assert self.sems is not None
    popped = self.nc._tile_sem_poison_stack.pop()
    assert popped is self._sem_poison
    self.nc.clear_and_free_semaphores(list(self.sems.allocated().values()))
    self.nc.all_engine_barrier()


tile.TileContext._drain_and_barrier = _patched_drain_and_barrier


def split_sync_waits(nc, max_waits=1):
    """Walrus codegen rejects >1 sync wait on several instruction encodings.
    Hoist excess waits onto same-engine no-ops placed just before."""
    import bass_rust
    try:
        funcs = list(nc.m.functions)
    except Exception:
        funcs = [nc.main_func]
    seen = 0
    for fn in funcs:
        for bb in fn.blocks:
            insts = bb.instructions
            new = []
            for ins in insts:
                si = ins.sync_info
                if si is not None and si.on_wait and len(si.on_wait) > max_waits:
                    waits = list(si.on_wait)
                    extra, keep = waits[:-max_waits], waits[-max_waits:]
                    for w in extra:
                        nop = bass_rust.InstNoOp(
                            name=f"I-waitsplit-{seen}", ins=[], outs=[])
                        seen += 1
                        nop.engine = ins.engine
                        nop.sync_info = mybir.SyncInfo(on_wait=[w], on_update=[])
                        new.append(nop)
                    del si.on_wait[:]
                    si.on_wait.extend(keep)
                new.append(ins)
            insts[:] = new
    return seen


def fix_gather_queues(nc, num_queues=4):
    """Tile's sem assigner hands DMASW sem lanes to Pool-engine DMA
    instructions round-robin in SCHEDULED order, ignoring queue_num. A sem
    lane must only ever be updated from one SWDGE queue, so rewrite each
    gather's queue_num as a pure function of its assigned lane."""
    gathers = []
    for fn in nc.m.functions:
        for bb in fn.blocks:
            for ins in bb.instructions:
                if type(ins).__name__ == "InstDMAGatherAnt":
                    si = ins.sync_info
                    assert si is not None and len(si.on_update) >= 1
                    gathers.append(ins)
    ids = [ins.sync_info.on_update[0].id for ins in gathers]
    id0 = min(ids)
    for ins, sid in zip(gathers, ids):
        lane = sid - id0
        assert 0 <= lane < 8, (sid, id0)
        ins.queue_num = lane % num_queues
    return len(gathers)


bf16 = ml_dtypes.bfloat16
P = 128          # partitions / tile edge
C = 8            # cores
D = 128          # hidden dim
NCLS = 2         # source-row classes
CLS_BLK = (32, 17)           # blocks per class (32*128=4096, 17*128=2176)
CLS_BASE = (0, 4096)
CLS_SZ = (4096, 2176)        # class-0 table = 8*4096 = 32768 rows (int16 max)
CB = 5           # dst blocks per gather chunk


# ---------------------------------------------------------------------------
# Host-side integer/index prep (sharding + metadata; no FP math on values)
# ---------------------------------------------------------------------------

def prep(edge_index, n_nodes):
    N = n_nodes
    npc = N // C
    assert npc * C == N
    B = (npc + P - 1) // P
    npad = B * P
    assert B == CLS_BLK[0] + CLS_BLK[1] and npad == CLS_SZ[0] + CLS_SZ[1]

    ei = np.asarray(edge_index)
    src_all = ei[0].astype(np.int64)
    dst_all = ei[1].astype(np.int64)
    # self-loops handled on-device via identity matmul; count in degree
    deg_all = np.bincount(dst_all, minlength=N) + 1

    own_s = src_all // npc
    loc_s = src_all - own_s * npc
    cls_all = (loc_s >= CLS_SZ[0]).astype(np.int64)
    # table rows are PARTITION-MAJOR within each core segment:
    # row = own*CLS_SZ + (loc%128)*(CLS_SZ//128) + loc//128, so the device
    # can write tables with contiguous per-partition DMA runs.
    nrb = (CLS_SZ[0] // P, CLS_SZ[1] // P)
    u0 = loc_s
    u1 = loc_s - CLS_BASE[1]
    row_all = np.where(cls_all == 0,
                       own_s * CLS_SZ[0] + (u0 % P) * nrb[0] + u0 // P,
                       own_s * CLS_SZ[1] + (u1 % P) * nrb[1] + u1 // P)

    owner_all = dst_all // npc
    per_core = []
    cnt = np.zeros((C, NCLS, B), dtype=np.int64)
    for c in range(C):
        m = owner_all == c
        r = row_all[m]
        k = cls_all[m]
        dloc = dst_all[m] - c * npc
        blk = dloc >> 7
        slot = dloc & 127
        order = np.lexsort((r, blk, k))
        r, k, blk, slot = r[order], k[order], blk[order], slot[order]
        per_core.append((r, k, blk, slot))
        for kk in range(NCLS):
            mk = k == kk
            cnt[c, kk] = np.bincount(blk[mk], minlength=B)

    # uniform tile counts: max over cores, per (class, block)
    T = [np.ceil(cnt[:, kk, :].max(axis=0) / P).astype(np.int64)
         for kk in range(NCLS)]
    tile_base = [np.concatenate([[0], np.cumsum(T[kk])]) for kk in range(NCLS)]
    T_total = [int(T[kk].sum()) for kk in range(NCLS)]

    idx = [np.zeros((C, T_total[kk] * P), dtype=np.int64) for kk in range(NCLS)]
    slots = [np.full((C, T_total[kk] * P), -1.0, dtype=np.float32)
             for kk in range(NCLS)]
    for c in range(C):
        r, k, blk, slot = per_core[c]
        for kk in range(NCLS):
            mk = k == kk
            rk, bk, sk = r[mk], blk[mk], slot[mk]
            bstart = np.concatenate([[0], np.cumsum(np.bincount(bk, minlength=B))])
            for b in range(B):
                e0, e1 = bstart[b], bstart[b + 1]
                o = tile_base[kk][b] * P
                idx[kk][c, o:o + (e1 - e0)] = rk[e0:e1]
                slots[kk][c, o:o + (e1 - e0)] = sk[e0:e1]

    deg = np.ones((C, P, B), dtype=np.float32)
    for c in range(C):
        dpad = np.ones(npad, dtype=np.float32)
        dpad[:npc] = deg_all[c * npc:(c + 1) * npc].astype(np.float32)
        deg[c] = dpad.reshape(B, P).T

    def pack16(a):
        # wrapped layout: element j -> [j % 16, j // 16], replicated to the
        # 8 Q7 cores' partition groups (128 partitions total)
        n = a.shape[1]
        w = a.reshape(a.shape[0], n // 16, 16).transpose(0, 2, 1).astype(np.int16)
        return np.tile(w, (1, 8, 1)).copy()

    chunks = []
    for b0 in range(0, B, CB):
        b1 = min(b0 + CB, B)
        chunks.append(dict(
            b0=b0, b1=b1,
            t0=[int(tile_base[kk][b0]) for kk in range(NCLS)],
            t1=[int(tile_base[kk][b1]) for kk in range(NCLS)],
        ))

    return dict(
        npc=npc, npad=npad, B=B,
        T=[T[kk].tolist() for kk in range(NCLS)],
        tile_base=[tile_base[kk].tolist() for kk in range(NCLS)],
        T_total=T_total, chunks=chunks,
        idx=[pack16(idx[kk]) for kk in range(NCLS)],
        slots=[slots[kk].reshape(C, T_total[kk], P).transpose(0, 2, 1).copy()
               for kk in range(NCLS)],
        deg=deg,
    )


# ---------------------------------------------------------------------------
# Device program (uniform across the 8 cores)
# ---------------------------------------------------------------------------

def build_program(meta):
    npad, B = meta['npad'], meta['B']
    T, tile_base, T_total = meta['T'], meta['tile_base'], meta['T_total']
    chunks = meta['chunks']
    f32 = mybir.dt.float32
    bf = mybir.dt.bfloat16
    max_ct = [max(ch['t1'][kk] - ch['t0'][kk] for ch in chunks)
              for kk in range(NCLS)]
    selmax = [max(T[kk]) for kk in range(NCLS)]

    nc = bacc.Bacc(None, target_bir_lowering=False, num_swdge_queues=4,
                   dynamic_dma_scratch_size=32768)
    # xpr: full x, bf16, partition-interleaved row-major:
    #   xpr[p, (c*B + j)*128 + f] = x[c*npc + j*128 + p, f]  (0 on pad rows)
    xpr_p = nc.declare_dram_parameter("xpr", [P, C * B * D], bf, isOutput=False)
    xT_p = nc.declare_dram_parameter("xT", [P, npad], bf, isOutput=False)
    degf_p = nc.declare_dram_parameter("degf", [P, C * B], f32, isOutput=False)
    w1_p = nc.declare_dram_parameter("w1", [P, D], f32, isOutput=False)
    w2_p = nc.declare_dram_parameter("w2", [P, D], f32, isOutput=False)
    rw_p = nc.declare_dram_parameter("resw", [P, D], f32, isOutput=False)
    cb_p = nc.declare_dram_parameter("convb", [2, D], f32, isOutput=False)
    rb_p = nc.declare_dram_parameter("resb", [1, D], f32, isOutput=False)
    deg_p = nc.declare_dram_parameter("deg", [P, B], f32, isOutput=False)
    idx_p = [nc.declare_dram_parameter(f"idx{kk}", [128, T_total[kk] * 8],
                                       mybir.dt.int16, isOutput=False)
             for kk in range(NCLS)]
    slot_p = [nc.declare_dram_parameter(f"slot{kk}", [P, T_total[kk]], bf,
                                        isOutput=False)
              for kk in range(NCLS)]
    warm_p = nc.declare_dram_parameter("warmidx", [128, 8], mybir.dt.int16,
                                       isOutput=False)
    # per-core own x segment (partition-interleaved), for the self-loop rows
    xprme_p = nc.declare_dram_parameter("xprme", [P, B * D], bf,
                                        isOutput=False)
    iota_p = nc.declare_dram_parameter("iota", [P, P], bf, isOutput=False)
    ident_p = nc.declare_dram_parameter("ident", [P, P], f32, isOutput=False)
    ones_p = nc.declare_dram_parameter("ones", [1, D], bf, isOutput=False)
    out_p = nc.declare_dram_parameter("out", [npad, D], f32, isOutput=True)

    # layer-2 x1~ shards (local) -> AllGathered class tables (shared)
    g_own = [nc.dram_tensor(f"g1o{kk}", [CLS_SZ[kk], D], bf)
             for kk in range(NCLS)]
    tbl = [[nc.dram_tensor(f"t0f{kk}", [C * CLS_SZ[kk], D], bf)
            for kk in range(NCLS)],
           [nc.dram_tensor(f"t1f{kk}", [C * CLS_SZ[kk], D], bf,
                           addr_space="Shared")
            for kk in range(NCLS)]]

    with tile.TileContext(nc) as tc, ExitStack() as ctx:
        const = ctx.enter_context(tc.tile_pool(name="const", bufs=1))
        gbuf0 = ctx.enter_context(tc.tile_pool(name="gbuf0", bufs=2))
        gbuf1 = ctx.enter_context(tc.tile_pool(name="gbuf1", bufs=5))
        work = ctx.enter_context(tc.tile_pool(name="work", bufs=3))
        outp = ctx.enter_context(tc.tile_pool(name="outp", bufs=3))
        psum = ctx.enter_context(tc.tile_pool(name="psum", bufs=2, space="PSUM"))

        # SWDGE queue assignment: DMASW sem lanes are handed out round-robin
        # (mod 8) over Pool-engine DMA instructions in emission order; keep
        # queue_num a pure function of that counter so each sem lane is only
        # ever updated from one queue.
        swq = [0]

        def next_q():
            q = (swq[0] % 8) % 3
            swq[0] += 1
            return q

        # ---- warm-up gather: absorb Q7 ucode first-call cost immediately
        warmidx = const.tile([128, 8], mybir.dt.int16)
        nc.sync.dma_start(out=warmidx[:], in_=warm_p[:, :])
        warmg = const.tile([P, 1, D], bf)
        nc.gpsimd.dma_gather(out_ap=warmg[:], in_ap=tbl[0][0][:, :],
                             idxs_ap=warmidx[:], num_idxs=128,
                             num_idxs_reg=128, elem_size=D,
                             single_packet=False, queue_num=next_q())

        # ---- build deps: full-degree dinv (bf16) ----
        degft = const.tile([P, C * B], f32)
        nc.sync.dma_start(out=degft[:], in_=degf_p[:, :])
        sdegf = const.tile([P, C * B], f32)
        nc.scalar.activation(out=sdegf[:], in_=degft[:],
                             func=mybir.ActivationFunctionType.Sqrt)
        dinvf = const.tile([P, C * B], f32)
        nc.vector.reciprocal(out=dinvf[:], in_=sdegf[:])
        dinvfb = const.tile([P, C * B], bf)
        nc.vector.tensor_copy(out=dinvfb[:], in_=dinvf[:])

        # ---- phase 1: build the layer-1 x~ tables (x~ = dinv * x).
        # Class-1 column slices of every core segment go first so layer-1
        # class-1 gathers can start early; class-0 follows while they run.
        bctx = ExitStack()
        bpool = bctx.enter_context(tc.tile_pool(name="bld", bufs=2))

        BSTEP = 17

        def emit_build(kk):
            nblk_c = CLS_BLK[kk]
            j0c = CLS_BASE[kk] // P             # first block of this class
            for o in range(C):
                for s0 in range(0, nblk_c, BSTEP):
                    nblk = min(BSTEP, nblk_c - s0)
                    j0 = j0c + s0
                    col0 = (o * B + j0) * D
                    xs = bpool.tile([P, BSTEP * D], bf, tag="xs")
                    nc.sync.dma_start(out=xs[:, :nblk * D],
                                      in_=xpr_p[:, col0:col0 + nblk * D])
                    xt = bpool.tile([P, BSTEP, D], bf, tag="xt")
                    nc.vector.tensor_tensor(
                        out=xt[:, :nblk, :],
                        in0=xs[:, :nblk * D].rearrange("p (j f) -> p j f", f=D),
                        in1=dinvfb[:, o * B + j0:o * B + j0 + nblk]
                            .rearrange("p (j o) -> p j o", o=1)
                            .to_broadcast([P, nblk, D]),
                        op=mybir.AluOpType.mult)
                    nrb = CLS_SZ[kk] // P
                    nc.scalar.dma_start(
                        out=tbl[0][kk]
                            .reshape([C, P, nrb, D])
                            .transpose([1, 0, 2, 3])[:, o, s0:s0 + nblk, :],
                        in_=xt[:, :nblk, :])

        with nc.named_scope("bld1"):
            emit_build(1)

        idxt = [None, None]
        i1 = const.tile([128, T_total[1] * 8], mybir.dt.int16, tag="idx1")
        nc.sync.dma_start(out=i1[:], in_=idx_p[1][:, :])
        idxt[1] = i1

        with nc.named_scope("bld0"):
            emit_build(0)

        # local dinv + own x~ rows (self-loop operand), before bld closes
        degt = const.tile([P, B], f32)
        nc.sync.dma_start(out=degt[:], in_=deg_p[:, :])
        sdeg = const.tile([P, B], f32)
        nc.scalar.activation(out=sdeg[:], in_=degt[:],
                             func=mybir.ActivationFunctionType.Sqrt)
        dinv = const.tile([P, B], f32)
        nc.vector.reciprocal(out=dinv[:], in_=sdeg[:])
        dinvb = const.tile([P, B], bf)
        nc.vector.tensor_copy(out=dinvb[:], in_=dinv[:])

        xtme = const.tile([P, B, D], bf)
        for s0 in range(0, B, BSTEP):
            nblk = min(BSTEP, B - s0)
            xs = bpool.tile([P, BSTEP * D], bf, tag="xs")
            nc.sync.dma_start(out=xs[:, :nblk * D],
                              in_=xprme_p[:, s0 * D:(s0 + nblk) * D])
            nc.vector.tensor_tensor(
                out=xtme[:, s0:s0 + nblk, :],
                in0=xs[:, :nblk * D].rearrange("p (j f) -> p j f", f=D),
                in1=dinvb[:, s0:s0 + nblk]
                    .rearrange("p (j o) -> p j o", o=1)
                    .to_broadcast([P, nblk, D]),
                op=mybir.AluOpType.mult)
        bctx.close()

        i0 = const.tile([128, T_total[0] * 8], mybir.dt.int16, tag="idx0")
        nc.scalar.dma_start(out=i0[:], in_=idx_p[0][:, :])
        idxt[0] = i0
        slots = [None, None]
        for kk in range(NCLS):
            st = const.tile([P, T_total[kk]], bf, tag=f"slot{kk}")
            nc.sync.dma_start(out=st[:], in_=slot_p[kk][:, :])
            slots[kk] = st

        # ---- remaining constants / persistent state ----
        xT = const.tile([P, npad], bf)
        nc.sync.dma_start(out=xT[:], in_=xT_p[:, :])
        x1T = const.tile([P, npad], bf)          # layer-1 output, transposed
        w1 = const.tile([P, D], f32)
        nc.sync.dma_start(out=w1[:], in_=w1_p[:, :])
        w1b = const.tile([P, D], bf)
        nc.vector.tensor_copy(out=w1b[:], in_=w1[:])
        w2f = const.tile([P, D], f32)
        nc.sync.dma_start(out=w2f[:], in_=w2_p[:, :])
        rwf = const.tile([P, D], f32)
        nc.sync.dma_start(out=rwf[:], in_=rw_p[:, :])
        w2b = const.tile([P, D], bf)
        nc.vector.tensor_copy(out=w2b[:], in_=w2f[:])
        rwb = const.tile([P, D], bf)
        nc.vector.tensor_copy(out=rwb[:], in_=rwf[:])
        wl = [w1b, w2b]

        rb = const.tile([1, D], f32)
        nc.sync.dma_start(out=rb[:], in_=rb_p[:, :])
        bcomb = []
        for l in range(2):
            cbl = const.tile([1, D], f32, tag=f"cb{l}")
            nc.sync.dma_start(out=cbl[:], in_=cb_p[l:l + 1, :])
            bc = const.tile([1, D], bf, tag=f"bcomb{l}")
            nc.vector.tensor_tensor(out=bc[:], in0=cbl[:], in1=rb[:],
                                    op=mybir.AluOpType.add)
            bcomb.append(bc)
        ones1 = const.tile([1, D], bf)
        nc.sync.dma_start(out=ones1[:], in_=ones_p[:, :])

        iota = const.tile([P, P], bf)
        nc.sync.dma_start(out=iota[:], in_=iota_p[:, :])
        ident = const.tile([P, P], f32)
        nc.sync.dma_start(out=ident[:], in_=ident_p[:, :])
        ident_bf = const.tile([P, P], bf)
        nc.vector.tensor_copy(out=ident_bf[:], in_=ident[:])

        x1me = const.tile([P, B, D], bf)        # layer-2 own x1~ rows

        def emit_gather(l, kk, ch, ci=-1):
            # split each chunk's gather across three SWDGE queues
            nt = ch['t1'][kk] - ch['t0'][kk]
            if nt == 0:
                return None
            t0 = ch['t0'][kk]
            pool = gbuf0 if kk == 0 else gbuf1
            gt = pool.tile([P, max_ct[kk], D], bf, tag=f"g{kk}")
            bounds = [t0 + (nt * q) // 2 for q in range(3)]
            with nc.named_scope(f"G{l}k{kk}c{ci}"):
                for q in range(2):
                    a, b2 = bounds[q], bounds[q + 1]
                    if b2 == a:
                        continue
                    nc.gpsimd.dma_gather(
                        out_ap=gt[:, a - t0:b2 - t0, :], in_ap=tbl[l][kk][:, :],
                        idxs_ap=idxt[kk][:, a * 8:b2 * 8],
                        num_idxs=(b2 - a) * P, num_idxs_reg=(b2 - a) * P,
                        elem_size=D, single_packet=False, queue_num=next_q())
            return gt

        def emit_agg(l, b, ch, gts):
            # pA: raw aggregate sum_e x~[src_e]  [slot, i]; then t1 on ACT
            pA = psum.tile([P, D], f32, tag="pA")
            own = xtme if l == 0 else x1me
            ntot = T[0][b] + T[1][b]
            nc.tensor.matmul(out=pA[:], lhsT=ident_bf[:], rhs=own[:, b, :],
                             start=True, stop=(ntot == 0))
            done = 0
            for kk in range(NCLS):
                ntk = T[kk][b]
                if ntk == 0:
                    continue
                tb = tile_base[kk][b]
                toff = tb - ch['t0'][kk]
                selb = work.tile([P, selmax[kk], P], bf, tag=f"sel{kk}")
                nc.vector.tensor_tensor(
                    out=selb[:, :ntk, :],
                    in0=slots[kk][:, tb:tb + ntk]
                        .rearrange("p (k o) -> p k o", o=1)
                        .to_broadcast([P, ntk, P]),
                    in1=iota[:].rearrange("p (o d) -> p o d", o=1)
                        .to_broadcast([P, ntk, P]),
                    op=mybir.AluOpType.is_equal)
                for t in range(ntk):
                    done += 1
                    nc.tensor.matmul(out=pA[:], lhsT=selb[:, t, :],
                                     rhs=gts[kk][:, toff + t, :],
                                     start=False, stop=(done == ntot))
            # t1 = dinv_dst * pA, bf16  [slot, i] (ACT, overlaps next agg)
            t1 = outp.tile([P, D], bf, tag="t1", bufs=6)
            nc.scalar.activation(out=t1[:], in_=pA[:],
                                 func=mybir.ActivationFunctionType.Copy,
                                 scale=dinv[:, b:b + 1])
            return t1

        def emit_mid(l, b, t1):
            # pT = t1^T [i, slot]; A = bf16 copy (DVE, overlaps next pT)
            pT = psum.tile([P, D], bf, tag="pT")
            nc.tensor.transpose(out=pT[:], in_=t1[:], identity=ident_bf[:])
            A = outp.tile([P, D], bf, tag="A", bufs=6)
            nc.vector.tensor_copy(out=A[:], in_=pT[:])
            return A

        def emit_fin(l, b, A):
            cs = slice(b * P, (b + 1) * P)
            kb = 0 if b < CLS_BLK[0] else 1
            lo = b * P - CLS_BASE[kb]
            # pB = A @ W_l + x_l @ resw + (conv_b + res_b)
            pB = psum.tile([P, D], f32, tag="pB")
            nc.tensor.matmul(out=pB[:], lhsT=A[:], rhs=wl[l][:],
                             start=True, stop=False)
            xl = xT if l == 0 else x1T
            nc.tensor.matmul(out=pB[:], lhsT=xl[:, cs], rhs=rwb[:],
                             start=False, stop=False)
            nc.tensor.matmul(out=pB[:], lhsT=ones1[:], rhs=bcomb[l][:],
                             start=False, stop=True)
            xo = outp.tile([P, D], f32, tag="xo", bufs=6)
            nc.scalar.activation(out=xo[:], in_=pB[:],
                                 func=mybir.ActivationFunctionType.Relu)
            if l == 0:
                # x1~ own rows: dinv * relu(pB), bf16 -> resident + shard
                # (shard rows are partition-major: row = p*nrb + rb)
                nc.scalar.activation(out=x1me[:, b, :], in_=xo[:],
                                     func=mybir.ActivationFunctionType.Copy,
                                     scale=dinv[:, b:b + 1])
                nrb = CLS_SZ[kb] // P
                rbb = lo // P
                nc.sync.dma_start(
                    out=g_own[kb].reshape([P, nrb, D])[:, rbb, :],
                    in_=x1me[:, b, :])
            else:
                nc.sync.dma_start(out=out_p[cs, :], in_=xo[:])
            return xo

        def emit_l1tail(b, xo):
            cs = slice(b * P, (b + 1) * P)
            pT2 = psum.tile([P, D], f32, tag="pT2")
            nc.tensor.transpose(out=pT2[:], in_=xo[:], identity=ident[:])
            nc.vector.tensor_copy(out=x1T[:, cs], in_=pT2[:])

        def emit_chunk(l, cj, ch, gts):
            blocks = list(range(ch['b0'], ch['b1']))
            t1s = [emit_agg(l, b, ch, gts) for b in blocks]
            As = [emit_mid(l, b, t1s[i]) for i, b in enumerate(blocks)]
            xos = [emit_fin(l, b, As[i]) for i, b in enumerate(blocks)]
            if l == 0:
                for i, b in enumerate(blocks):
                    emit_l1tail(b, xos[i])

        # ---- layer 1, chunks processed in REVERSE order: class-1 blocks
        # (32..48) complete ~40% in, so the class-1 x1~ AllGather (ag2b)
        # fires early and is fully hidden; the class-0 AllGather (ag2a)
        # fires at layer-1 end and hides behind layer-2's class-1 gather
        # lead. k1 gathers lead blocks by 3 chunks, k0 by 2. ----
        nch = len(chunks)
        with nc.named_scope("layer1"):
            l1_gts = {c: [None, None] for c in range(nch)}
            ord1 = list(reversed(range(nch)))

            def finish_chunk1(cj):
                emit_chunk(0, cj, chunks[cj], l1_gts[cj])
                if chunks[cj]['b0'] <= CLS_BLK[0] < chunks[cj]['b1']:
                    # blocks 32..48 (class-1 x1~ rows) all written now
                    with nc.named_scope("ag2b"):
                        nc.gpsimd.collective_compute(
                            "AllGather", mybir.AluOpType.bypass,
                            replica_groups=[list(range(C))],
                            ins=[g_own[1][:, :]], outs=[tbl[1][1][:, :]])

            for i in range(nch + 4):
                if i < nch:
                    cj = ord1[i]
                    l1_gts[cj][1] = emit_gather(0, 1, chunks[cj], cj)
                if 3 <= i < nch + 3:
                    ck = ord1[i - 3]
                    l1_gts[ck][0] = emit_gather(0, 0, chunks[ck], ck)
                if i >= 4:
                    finish_chunk1(ord1[i - 4])
        # ---- layer 2 (forward order): the first three k1 gather groups are
        # emitted BEFORE ag2a so the in-order Pool stream has work queued
        # while the class-0 AllGather completes; k0 gathers lead blocks
        # by 1 chunk ----
        gts_map = {c: [None, None] for c in range(nch)}
        with nc.named_scope("layer2"):
            for i in range(3):
                gts_map[i][1] = emit_gather(1, 1, chunks[i], i)
        with nc.named_scope("ag2a"):
            nc.gpsimd.collective_compute(
                "AllGather", mybir.AluOpType.bypass,
                replica_groups=[list(range(C))],
                ins=[g_own[0][:, :]], outs=[tbl[1][0][:, :]])
        with nc.named_scope("layer2"):
            for i in range(3, nch + 4):
                if i < nch:
                    gts_map[i][1] = emit_gather(1, 1, chunks[i], i)
                if i - 3 < nch:
                    gts_map[i - 3][0] = emit_gather(1, 0, chunks[i - 3], i - 3)
                if i >= 4:
                    emit_chunk(1, i - 4, chunks[i - 4], gts_map[i - 4])
    return nc


# ---------------------------------------------------------------------------
# Entry point
# ---------------------------------------------------------------------------

def make_inputs(x, conv_w, conv_b, res_w, res_b, meta):
    npc, npad, B = meta['npc'], meta['npad'], meta['B']
    iota = np.tile(np.arange(P, dtype=np.float32), (P, 1)).astype(bf16)
    warm = np.zeros((128, 8), dtype=np.int16)
    xf = np.asarray(x, dtype=np.float32)
    # xpr: [p, (c*B + j)*128 + f] = x[c*npc + j*128 + p, f]
    xpad = np.zeros((C, npad, D), dtype=np.float32)
    for c in range(C):
        xpad[c, :npc] = xf[c * npc:(c + 1) * npc]
    # [C, B, P, D] -> [P, C, B, D]
    xpr = xpad.reshape(C, B, P, D).transpose(2, 0, 1, 3) \
              .reshape(P, C * B * D).astype(bf16)
    degf_full = np.concatenate([meta['deg'][o] for o in range(C)], axis=1)
    in_maps = []
    for c in range(C):
        xT = np.zeros((P, npad), dtype=bf16)
        xT[:, :npc] = xf[c * npc:(c + 1) * npc].T.astype(bf16)
        xprme = xpad[c].reshape(B, P, D).transpose(1, 0, 2) \
                       .reshape(P, B * D).astype(bf16)
        in_maps.append({
            "xpr": xpr,
            "xprme": xprme,
            "xT": xT,
            "degf": degf_full,
            "w1": np.asarray(conv_w[0], dtype=np.float32),
            "w2": np.asarray(conv_w[1], dtype=np.float32),
            "resw": np.asarray(res_w, dtype=np.float32),
            "convb": np.asarray(conv_b, dtype=np.float32),
            "resb": np.asarray(res_b, dtype=np.float32).reshape(1, D),
            "deg": meta['deg'][c],
            "idx0": meta['idx'][0][c],
            "idx1": meta['idx'][1][c],
            "slot0": meta['slots'][0][c].astype(bf16),
            "slot1": meta['slots'][1][c].astype(bf16),
            "warmidx": warm,
            "iota": iota,
            "ident": np.eye(P, dtype=np.float32),
            "ones": np.ones((1, D), dtype=np.float32).astype(bf16),
        })
    return in_maps


def run(x, edge_index, conv_w, conv_b, res_w, res_b, trace=False, trace_kwargs=None):
    N = x.shape[0]
    meta = prep(edge_index, N)
    nc = build_program(meta)
    nc.compile()
    fix_gather_queues(nc)
    split_sync_waits(nc)
    in_maps = make_inputs(x, conv_w, conv_b, res_w, res_b, meta)
    res = run_bass_kernel_spmd(nc, in_maps, list(range(C)), trace=trace,
                               **(trace_kwargs or {}))
    npc = meta['npc']
    out = np.concatenate([np.asarray(res.results[c]["out"])[:npc]
                          for c in range(C)], axis=0)
    return out.astype(np.float32), res


def kernel(x, edge_index, conv_w, conv_b, res_w, res_b):
    out, _ = run(x, edge_index, conv_w, conv_b, res_w, res_b, trace=False)
    return out


# revision 38
# speedup vs baseline: 1.1668x; 1.1668x over previous
"""GCN 2-layer message-passing kernel for Trainium2 (8 NeuronCores, Bass/Tile).

Strategy (graph/data parallel), v2:
  - Nodes partitioned into 8 contiguous ranges (6250 per core, padded 6272).
  - Host does INTEGER/index prep only. All FP math runs on device.
  - KEY CHANGE vs v1: gather and the dense transform commute
    (sum_e sel_e * (xW) = (sum_e sel_e * x) W), so the gather tables hold
    RAW dinv-scaled features (x~ = dinv * x for layer 1, x1~ = dinv * x1
    for layer 2) instead of the transformed g = dinv * (x @ W). The @W
    moves into a tiny per-block epilogue (transpose + 3 chained matmuls).
    This deletes the 392-matmul/392-ACT-copy per-core table build and the
    ~90us gather hole it caused; layer-1 gathers start ~40us in, right
    after a single DVE broadcast-multiply builds the x~ table.
  - Per layer, per chunk of 5 destination blocks: one dma_gather per class
    (split across 3 SWDGE queues) fetches all message rows; per 128-edge
    tile a 0/1 selection matrix (DVE is_equal vs iota) scatter-reduces raw
    messages into the block's PSUM accumulator via one PE matmul;
    self-loops via an identity matmul on the own-block rows (DMA'd from
    the table); dst-degree scaling via ACT per-partition scale; then the
    epilogue transform: PE transpose -> SBUF -> (agg @ W + x @ resw +
    bias) chained into a second PSUM bank -> relu.
  - Layer-2 x1~ rows are produced inside the layer-1 block loop and
    AllGathered class-by-class as soon as ready (class split keeps tables
    under the int16 index limit).
  - A tiny warm-up dma_gather at t=0 absorbs the Q7 ucode first-call cost.

kernel(**inputs) takes FULL inputs and returns the FULL [50000, 128]
float32 output.
"""
import sys
from contextlib import ExitStack

import numpy as np

if '/opt/trn_rl_repo' not in sys.path:
    sys.path.insert(0, '/opt/trn_rl_repo')

import ml_dtypes

from concourse import bacc, mybir, tile
from concourse.bass_utils import run_bass_kernel_spmd
from concourse.vector_clock import ScopedClock


def _patched_drain_and_barrier(self, tick_clock, wait_clock):
    """Split the kernel-tail drain's sem waits across single-wait drains:
    walrus's NO_STRUCT codegen rejects >1 sync wait on InstDrain."""
    drain_inst = self.nc.sync.drain()
    wait_clock.add_sem_waits(drain_inst.ins,
                             ScopedClock({None: tick_clock.global_clock}))
    si = drain_inst.ins.sync_info
    if si is not None and si.on_wait is not None and len(si.on_wait) > 1:
        waits = list(si.on_wait)
        del si.on_wait[1:]
        for w in waits[1:]:
            d2 = self.nc.sync.drain()
            si2 = d2.ins.sync_info
            if si2 is None:
                d2.ins.sync_info = mybir.SyncInfo(on_wait=[w], on_update=[])
            else:
                si2.on_wait.append(w)
    self.nc.all_engine_barrier()
    assert self.sems is not None
    popped = self.nc._tile_sem_poison_stack.pop()
    assert popped is self._sem_poison
    self.nc.clear_and_free_semaphores(list(self.sems.allocated().values()))
    self.nc.all_engine_barrier()


tile.TileContext._drain_and_barrier = _patched_drain_and_barrier


def split_sync_waits(nc, max_waits=1):
    """Walrus codegen rejects >1 sync wait on several instruction encodings.
    Hoist excess waits onto same-engine no-ops placed just before."""
    import bass_rust
    try:
        funcs = list(nc.m.functions)
    except Exception:
        funcs = [nc.main_func]
    seen = 0
    for fn in funcs:
        for bb in fn.blocks:
            insts = bb.instructions
            new = []
            for ins in insts:
                si = ins.sync_info
                if si is not None and si.on_wait and len(si.on_wait) > max_waits:
                    waits = list(si.on_wait)
                    extra, keep = waits[:-max_waits], waits[-max_waits:]
                    for w in extra:
                        nop = bass_rust.InstNoOp(
                            name=f"I-waitsplit-{seen}", ins=[], outs=[])
                        seen += 1
                        nop.engine = ins.engine
                        nop.sync_info = mybir.SyncInfo(on_wait=[w], on_update=[])
                        new.append(nop)
                    del si.on_wait[:]
                    si.on_wait.extend(keep)
                new.append(ins)
            insts[:] = new
    return seen


def fix_gather_queues(nc, num_queues=4):
    """Tile's sem assigner hands DMASW sem lanes to Pool-engine DMA
    instructions round-robin in SCHEDULED order, ignoring queue_num. A sem
    lane must only ever be updated from one SWDGE queue, so rewrite each
    gather's queue_num as a pure function of its assigned lane."""
    gathers = []
    for fn in nc.m.functions:
        for bb in fn.blocks:
            for ins in bb.instructions:
                if type(ins).__name__ == "InstDMAGatherAnt":
                    si = ins.sync_info
                    assert si is not None and len(si.on_update) >= 1
                    gathers.append(ins)
    ids = [ins.sync_info.on_update[0].id for ins in gathers]
    id0 = min(ids)
    for ins, sid in zip(gathers, ids):
        lane = sid - id0
        assert 0 <= lane < 8, (sid, id0)
        ins.queue_num = lane % num_queues
    return len(gathers)


bf16 = ml_dtypes.bfloat16
P = 128          # partitions / tile edge
C = 8            # cores
D = 128          # hidden dim
NCLS = 2         # source-row classes
CLS_BLK = (32, 17)           # blocks per class (32*128=4096, 17*128=2176)
CLS_BASE = (0, 4096)
CLS_SZ = (4096, 2176)        # class-0 table = 8*4096 = 32768 rows (int16 max)
CB = 5           # dst blocks per gather chunk


# ---------------------------------------------------------------------------
# Host-side integer/index prep (sharding + metadata; no FP math on values)
# ---------------------------------------------------------------------------

def prep(edge_index, n_nodes):
    N = n_nodes
    npc = N // C
    assert npc * C == N
    B = (npc + P - 1) // P
    npad = B * P
    assert B == CLS_BLK[0] + CLS_BLK[1] and npad == CLS_SZ[0] + CLS_SZ[1]

    ei = np.asarray(edge_index)
    src_all = ei[0].astype(np.int64)
    dst_all = ei[1].astype(np.int64)
    # self-loops handled on-device via identity matmul; count in degree
    deg_all = np.bincount(dst_all, minlength=N) + 1

    own_s = src_all // npc
    loc_s = src_all - own_s * npc
    cls_all = (loc_s >= CLS_SZ[0]).astype(np.int64)
    # table rows are PARTITION-MAJOR within each core segment:
    # row = own*CLS_SZ + (loc%128)*(CLS_SZ//128) + loc//128, so the device
    # can write tables with contiguous per-partition DMA runs.
    nrb = (CLS_SZ[0] // P, CLS_SZ[1] // P)
    u0 = loc_s
    u1 = loc_s - CLS_BASE[1]
    row_all = np.where(cls_all == 0,
                       own_s * CLS_SZ[0] + (u0 % P) * nrb[0] + u0 // P,
                       own_s * CLS_SZ[1] + (u1 % P) * nrb[1] + u1 // P)

    owner_all = dst_all // npc
    per_core = []
    cnt = np.zeros((C, NCLS, B), dtype=np.int64)
    for c in range(C):
        m = owner_all == c
        r = row_all[m]
        k = cls_all[m]
        dloc = dst_all[m] - c * npc
        blk = dloc >> 7
        slot = dloc & 127
        order = np.lexsort((r, blk, k))
        r, k, blk, slot = r[order], k[order], blk[order], slot[order]
        per_core.append((r, k, blk, slot))
        for kk in range(NCLS):
            mk = k == kk
            cnt[c, kk] = np.bincount(blk[mk], minlength=B)

    # uniform tile counts: max over cores, per (class, block)
    T = [np.ceil(cnt[:, kk, :].max(axis=0) / P).astype(np.int64)
         for kk in range(NCLS)]
    tile_base = [np.concatenate([[0], np.cumsum(T[kk])]) for kk in range(NCLS)]
    T_total = [int(T[kk].sum()) for kk in range(NCLS)]

    idx = [np.zeros((C, T_total[kk] * P), dtype=np.int64) for kk in range(NCLS)]
    slots = [np.full((C, T_total[kk] * P), -1.0, dtype=np.float32)
             for kk in range(NCLS)]
    for c in range(C):
        r, k, blk, slot = per_core[c]
        for kk in range(NCLS):
            mk = k == kk
            rk, bk, sk = r[mk], blk[mk], slot[mk]
            bstart = np.concatenate([[0], np.cumsum(np.bincount(bk, minlength=B))])
            for b in range(B):
                e0, e1 = bstart[b], bstart[b + 1]
                o = tile_base[kk][b] * P
                idx[kk][c, o:o + (e1 - e0)] = rk[e0:e1]
                slots[kk][c, o:o + (e1 - e0)] = sk[e0:e1]

    deg = np.ones((C, P, B), dtype=np.float32)
    for c in range(C):
        dpad = np.ones(npad, dtype=np.float32)
        dpad[:npc] = deg_all[c * npc:(c + 1) * npc].astype(np.float32)
        deg[c] = dpad.reshape(B, P).T

    def pack16(a):
        # wrapped layout: element j -> [j % 16, j // 16], replicated to the
        # 8 Q7 cores' partition groups (128 partitions total)
        n = a.shape[1]
        w = a.reshape(a.shape[0], n // 16, 16).transpose(0, 2, 1).astype(np.int16)
        return np.tile(w, (1, 8, 1)).copy()

    chunks = []
    for b0 in range(0, B, CB):
        b1 = min(b0 + CB, B)
        chunks.append(dict(
            b0=b0, b1=b1,
            t0=[int(tile_base[kk][b0]) for kk in range(NCLS)],
            t1=[int(tile_base[kk][b1]) for kk in range(NCLS)],
        ))

    return dict(
        npc=npc, npad=npad, B=B,
        T=[T[kk].tolist() for kk in range(NCLS)],
        tile_base=[tile_base[kk].tolist() for kk in range(NCLS)],
        T_total=T_total, chunks=chunks,
        idx=[pack16(idx[kk]) for kk in range(NCLS)],
        slots=[slots[kk].reshape(C, T_total[kk], P).transpose(0, 2, 1).copy()
               for kk in range(NCLS)],
        deg=deg,
    )


# ---------------------------------------------------------------------------
# Device program (uniform across the 8 cores)
# ---------------------------------------------------------------------------

def build_program(meta):
    npad, B = meta['npad'], meta['B']
    T, tile_base, T_total = meta['T'], meta['tile_base'], meta['T_total']
    chunks = meta['chunks']
    f32 = mybir.dt.float32
    bf = mybir.dt.bfloat16
    max_ct = [max(ch['t1'][kk] - ch['t0'][kk] for ch in chunks)
              for kk in range(NCLS)]
    selmax = [max(T[kk]) for kk in range(NCLS)]

    nc = bacc.Bacc(None, target_bir_lowering=False, num_swdge_queues=4,
                   dynamic_dma_scratch_size=32768)
    # xpr: full x, bf16, partition-interleaved row-major:
    #   xpr[p, (c*B + j)*128 + f] = x[c*npc + j*128 + p, f]  (0 on pad rows)
    xpr_p = nc.declare_dram_parameter("xpr", [P, C * B * D], bf, isOutput=False)
    xT_p = nc.declare_dram_parameter("xT", [P, npad], bf, isOutput=False)
    degf_p = nc.declare_dram_parameter("degf", [P, C * B], f32, isOutput=False)
    w1_p = nc.declare_dram_parameter("w1", [P, D], f32, isOutput=False)
    w2_p = nc.declare_dram_parameter("w2", [P, D], f32, isOutput=False)
    rw_p = nc.declare_dram_parameter("resw", [P, D], f32, isOutput=False)
    cb_p = nc.declare_dram_parameter("convb", [2, D], f32, isOutput=False)
    rb_p = nc.declare_dram_parameter("resb", [1, D], f32, isOutput=False)
    deg_p = nc.declare_dram_parameter("deg", [P, B], f32, isOutput=False)
    idx_p = [nc.declare_dram_parameter(f"idx{kk}", [128, T_total[kk] * 8],
                                       mybir.dt.int16, isOutput=False)
             for kk in range(NCLS)]
    slot_p = [nc.declare_dram_parameter(f"slot{kk}", [P, T_total[kk]], bf,
                                        isOutput=False)
              for kk in range(NCLS)]
    warm_p = nc.declare_dram_parameter("warmidx", [128, 8], mybir.dt.int16,
                                       isOutput=False)
    # per-core own x segment (partition-interleaved), for the self-loop rows
    xprme_p = nc.declare_dram_parameter("xprme", [P, B * D], bf,
                                        isOutput=False)
    iota_p = nc.declare_dram_parameter("iota", [P, P], bf, isOutput=False)
    ident_p = nc.declare_dram_parameter("ident", [P, P], f32, isOutput=False)
    ones_p = nc.declare_dram_parameter("ones", [1, D], bf, isOutput=False)
    out_p = nc.declare_dram_parameter("out", [npad, D], f32, isOutput=True)

    # layer-2 x1~ shards (local) -> AllGathered class tables (shared)
    g_own = [nc.dram_tensor(f"g1o{kk}", [CLS_SZ[kk], D], bf)
             for kk in range(NCLS)]
    tbl = [[nc.dram_tensor(f"t0f{kk}", [C * CLS_SZ[kk], D], bf)
            for kk in range(NCLS)],
           [nc.dram_tensor(f"t1f{kk}", [C * CLS_SZ[kk], D], bf,
                           addr_space="Shared")
            for kk in range(NCLS)]]

    with tile.TileContext(nc) as tc, ExitStack() as ctx:
        const = ctx.enter_context(tc.tile_pool(name="const", bufs=1))
        gbuf0 = ctx.enter_context(tc.tile_pool(name="gbuf0", bufs=2))
        gbuf1 = ctx.enter_context(tc.tile_pool(name="gbuf1", bufs=5))
        work = ctx.enter_context(tc.tile_pool(name="work", bufs=3))
        outp = ctx.enter_context(tc.tile_pool(name="outp", bufs=3))
        psum = ctx.enter_context(tc.tile_pool(name="psum", bufs=2, space="PSUM"))

        # SWDGE queue assignment: DMASW sem lanes are handed out round-robin
        # (mod 8) over Pool-engine DMA instructions in emission order; keep
        # queue_num a pure function of that counter so each sem lane is only
        # ever updated from one queue.
        swq = [0]

        def next_q():
            q = (swq[0] % 8) % 3
            swq[0] += 1
            return q

        # ---- warm-up gather: absorb Q7 ucode first-call cost immediately
        warmidx = const.tile([128, 8], mybir.dt.int16)
        nc.sync.dma_start(out=warmidx[:], in_=warm_p[:, :])
        warmg = const.tile([P, 1, D], bf)
        nc.gpsimd.dma_gather(out_ap=warmg[:], in_ap=tbl[0][0][:, :],
                             idxs_ap=warmidx[:], num_idxs=128,
                             num_idxs_reg=128, elem_size=D,
                             single_packet=False, queue_num=next_q())

        # ---- build deps: full-degree dinv (bf16) ----
        degft = const.tile([P, C * B], f32)
        nc.sync.dma_start(out=degft[:], in_=degf_p[:, :])
        sdegf = const.tile([P, C * B], f32)
        nc.scalar.activation(out=sdegf[:], in_=degft[:],
                             func=mybir.ActivationFunctionType.Sqrt)
        dinvf = const.tile([P, C * B], f32)
        nc.vector.reciprocal(out=dinvf[:], in_=sdegf[:])
        dinvfb = const.tile([P, C * B], bf)
        nc.vector.tensor_copy(out=dinvfb[:], in_=dinvf[:])

        # ---- phase 1: build the layer-1 x~ tables (x~ = dinv * x).
        # Class-1 column slices of every core segment go first so layer-1
        # class-1 gathers can start early; class-0 follows while they run.
        bctx = ExitStack()
        bpool = bctx.enter_context(tc.tile_pool(name="bld", bufs=2))

        BSTEP = 17

        def emit_build(kk):
            nblk_c = CLS_BLK[kk]
            j0c = CLS_BASE[kk] // P             # first block of this class
            for o in range(C):
                for s0 in range(0, nblk_c, BSTEP):
                    nblk = min(BSTEP, nblk_c - s0)
                    j0 = j0c + s0
                    col0 = (o * B + j0) * D
                    xs = bpool.tile([P, BSTEP * D], bf, tag="xs")
                    nc.sync.dma_start(out=xs[:, :nblk * D],
                                      in_=xpr_p[:, col0:col0 + nblk * D])
                    xt = bpool.tile([P, BSTEP, D], bf, tag="xt")
                    nc.vector.tensor_tensor(
                        out=xt[:, :nblk, :],
                        in0=xs[:, :nblk * D].rearrange("p (j f) -> p j f", f=D),
                        in1=dinvfb[:, o * B + j0:o * B + j0 + nblk]
                            .rearrange("p (j o) -> p j o", o=1)
                            .to_broadcast([P, nblk, D]),
                        op=mybir.AluOpType.mult)
                    nrb = CLS_SZ[kk] // P
                    nc.scalar.dma_start(
                        out=tbl[0][kk]
                            .reshape([C, P, nrb, D])
                            .transpose([1, 0, 2, 3])[:, o, s0:s0 + nblk, :],
                        in_=xt[:, :nblk, :])

        with nc.named_scope("bld1"):
            emit_build(1)

        idxt = [None, None]
        i1 = const.tile([128, T_total[1] * 8], mybir.dt.int16, tag="idx1")
        nc.sync.dma_start(out=i1[:], in_=idx_p[1][:, :])
        idxt[1] = i1

        with nc.named_scope("bld0"):
            emit_build(0)

        # local dinv + own x~ rows (self-loop operand), before bld closes
        degt = const.tile([P, B], f32)
        nc.sync.dma_start(out=degt[:], in_=deg_p[:, :])
        sdeg = const.tile([P, B], f32)
        nc.scalar.activation(out=sdeg[:], in_=degt[:],
                             func=mybir.ActivationFunctionType.Sqrt)
        dinv = const.tile([P, B], f32)
        nc.vector.reciprocal(out=dinv[:], in_=sdeg[:])
        dinvb = const.tile([P, B], bf)
        nc.vector.tensor_copy(out=dinvb[:], in_=dinv[:])

        xtme = const.tile([P, B, D], bf)
        for s0 in range(0, B, BSTEP):
            nblk = min(BSTEP, B - s0)
            xs = bpool.tile([P, BSTEP * D], bf, tag="xs")
            nc.sync.dma_start(out=xs[:, :nblk * D],
                              in_=xprme_p[:, s0 * D:(s0 + nblk) * D])
            nc.vector.tensor_tensor(
                out=xtme[:, s0:s0 + nblk, :],
                in0=xs[:, :nblk * D].rearrange("p (j f) -> p j f", f=D),
                in1=dinvb[:, s0:s0 + nblk]
                    .rearrange("p (j o) -> p j o", o=1)
                    .to_broadcast([P, nblk, D]),
                op=mybir.AluOpType.mult)
        bctx.close()

        i0 = const.tile([128, T_total[0] * 8], mybir.dt.int16, tag="idx0")
        nc.scalar.dma_start(out=i0[:], in_=idx_p[0][:, :])
        idxt[0] = i0
        slots = [None, None]
        for kk in range(NCLS):
            st = const.tile([P, T_total[kk]], bf, tag=f"slot{kk}")
            nc.sync.dma_start(out=st[:], in_=slot_p[kk][:, :])
            slots[kk] = st

        # ---- remaining constants / persistent state ----
        xT = const.tile([P, npad], bf)
        nc.sync.dma_start(out=xT[:], in_=xT_p[:, :])
        x1T = const.tile([P, npad], bf)          # layer-1 output, transposed
        w1 = const.tile([P, D], f32)
        nc.sync.dma_start(out=w1[:], in_=w1_p[:, :])
        w1b = const.tile([P, D], bf)
        nc.vector.tensor_copy(out=w1b[:], in_=w1[:])
        w2f = const.tile([P, D], f32)
        nc.sync.dma_start(out=w2f[:], in_=w2_p[:, :])
        rwf = const.tile([P, D], f32)
        nc.sync.dma_start(out=rwf[:], in_=rw_p[:, :])
        w2b = const.tile([P, D], bf)
        nc.vector.tensor_copy(out=w2b[:], in_=w2f[:])
        rwb = const.tile([P, D], bf)
        nc.vector.tensor_copy(out=rwb[:], in_=rwf[:])
        wl = [w1b, w2b]

        rb = const.tile([1, D], f32)
        nc.sync.dma_start(out=rb[:], in_=rb_p[:, :])
        bcomb = []
        for l in range(2):
            cbl = const.tile([1, D], f32, tag=f"cb{l}")
            nc.sync.dma_start(out=cbl[:], in_=cb_p[l:l + 1, :])
            bc = const.tile([1, D], bf, tag=f"bcomb{l}")
            nc.vector.tensor_tensor(out=bc[:], in0=cbl[:], in1=rb[:],
                                    op=mybir.AluOpType.add)
            bcomb.append(bc)
        ones1 = const.tile([1, D], bf)
        nc.sync.dma_start(out=ones1[:], in_=ones_p[:, :])

        iota = const.tile([P, P], bf)
        nc.sync.dma_start(out=iota[:], in_=iota_p[:, :])
        ident = const.tile([P, P], f32)
        nc.sync.dma_start(out=ident[:], in_=ident_p[:, :])
        ident_bf = const.tile([P, P], bf)
        nc.vector.tensor_copy(out=ident_bf[:], in_=ident[:])

        x1me = const.tile([P, B, D], bf)        # layer-2 own x1~ rows

        def emit_gather(l, kk, ch, ci=-1):
            # split each chunk's gather across three SWDGE queues
            nt = ch['t1'][kk] - ch['t0'][kk]
            if nt == 0:
                return None
            t0 = ch['t0'][kk]
            pool = gbuf0 if kk == 0 else gbuf1
            gt = pool.tile([P, max_ct[kk], D], bf, tag=f"g{kk}")
            bounds = [t0 + (nt * q) // 4 for q in range(5)]
            with nc.named_scope(f"G{l}k{kk}c{ci}"):
                for q in range(4):
                    a, b2 = bounds[q], bounds[q + 1]
                    if b2 == a:
                        continue
                    nc.gpsimd.dma_gather(
                        out_ap=gt[:, a - t0:b2 - t0, :], in_ap=tbl[l][kk][:, :],
                        idxs_ap=idxt[kk][:, a * 8:b2 * 8],
                        num_idxs=(b2 - a) * P, num_idxs_reg=(b2 - a) * P,
                        elem_size=D, single_packet=False, queue_num=next_q())
            return gt

        def emit_agg(l, b, ch, gts):
            # pA: raw aggregate sum_e x~[src_e]  [slot, i]; then t1 on ACT
            pA = psum.tile([P, D], f32, tag="pA")
            own = xtme if l == 0 else x1me
            ntot = T[0][b] + T[1][b]
            nc.tensor.matmul(out=pA[:], lhsT=ident_bf[:], rhs=own[:, b, :],
                             start=True, stop=(ntot == 0))
            done = 0
            for kk in range(NCLS):
                ntk = T[kk][b]
                if ntk == 0:
                    continue
                tb = tile_base[kk][b]
                toff = tb - ch['t0'][kk]
                selb = work.tile([P, selmax[kk], P], bf, tag=f"sel{kk}")
                nc.vector.tensor_tensor(
                    out=selb[:, :ntk, :],
                    in0=slots[kk][:, tb:tb + ntk]
                        .rearrange("p (k o) -> p k o", o=1)
                        .to_broadcast([P, ntk, P]),
                    in1=iota[:].rearrange("p (o d) -> p o d", o=1)
                        .to_broadcast([P, ntk, P]),
                    op=mybir.AluOpType.is_equal)
                for t in range(ntk):
                    done += 1
                    nc.tensor.matmul(out=pA[:], lhsT=selb[:, t, :],
                                     rhs=gts[kk][:, toff + t, :],
                                     start=False, stop=(done == ntot))
            # t1 = dinv_dst * pA, bf16  [slot, i] (ACT, overlaps next agg)
            t1 = outp.tile([P, D], bf, tag="t1", bufs=6)
            nc.scalar.activation(out=t1[:], in_=pA[:],
                                 func=mybir.ActivationFunctionType.Copy,
                                 scale=dinv[:, b:b + 1])
            return t1

        def emit_mid(l, b, t1):
            # pT = t1^T [i, slot]; A = bf16 copy (DVE, overlaps next pT)
            pT = psum.tile([P, D], bf, tag="pT")
            nc.tensor.transpose(out=pT[:], in_=t1[:], identity=ident_bf[:])
            A = outp.tile([P, D], bf, tag="A", bufs=6)
            nc.vector.tensor_copy(out=A[:], in_=pT[:])
            return A

        def emit_fin(l, b, A):
            cs = slice(b * P, (b + 1) * P)
            kb = 0 if b < CLS_BLK[0] else 1
            lo = b * P - CLS_BASE[kb]
            # pB = A @ W_l + x_l @ resw + (conv_b + res_b)
            pB = psum.tile([P, D], f32, tag="pB")
            nc.tensor.matmul(out=pB[:], lhsT=A[:], rhs=wl[l][:],
                             start=True, stop=False)
            xl = xT if l == 0 else x1T
            nc.tensor.matmul(out=pB[:], lhsT=xl[:, cs], rhs=rwb[:],
                             start=False, stop=False)
            nc.tensor.matmul(out=pB[:], lhsT=ones1[:], rhs=bcomb[l][:],
                             start=False, stop=True)
            xo = outp.tile([P, D], f32, tag="xo", bufs=6)
            nc.scalar.activation(out=xo[:], in_=pB[:],
                                 func=mybir.ActivationFunctionType.Relu)
            if l == 0:
                # x1~ own rows: dinv * relu(pB), bf16 -> resident + shard
                # (shard rows are partition-major: row = p*nrb + rb)
                nc.scalar.activation(out=x1me[:, b, :], in_=xo[:],
                                     func=mybir.ActivationFunctionType.Copy,
                                     scale=dinv[:, b:b + 1])
                nrb = CLS_SZ[kb] // P
                rbb = lo // P
                nc.sync.dma_start(
                    out=g_own[kb].reshape([P, nrb, D])[:, rbb, :],
                    in_=x1me[:, b, :])
            else:
                nc.sync.dma_start(out=out_p[cs, :], in_=xo[:])
            return xo

        def emit_l1tail(b, xo):
            cs = slice(b * P, (b + 1) * P)
            pT2 = psum.tile([P, D], f32, tag="pT2")
            nc.tensor.transpose(out=pT2[:], in_=xo[:], identity=ident[:])
            nc.vector.tensor_copy(out=x1T[:, cs], in_=pT2[:])

        def emit_chunk(l, cj, ch, gts):
            blocks = list(range(ch['b0'], ch['b1']))
            t1s = [emit_agg(l, b, ch, gts) for b in blocks]
            As = [emit_mid(l, b, t1s[i]) for i, b in enumerate(blocks)]
            xos = [emit_fin(l, b, As[i]) for i, b in enumerate(blocks)]
            if l == 0:
                for i, b in enumerate(blocks):
                    emit_l1tail(b, xos[i])

        # ---- layer 1, chunks processed in REVERSE order: class-1 blocks
        # (32..48) complete ~40% in, so the class-1 x1~ AllGather (ag2b)
        # fires early and is fully hidden; the class-0 AllGather (ag2a)
        # fires at layer-1 end and hides behind layer-2's class-1 gather
        # lead. k1 gathers lead blocks by 3 chunks, k0 by 2. ----
        nch = len(chunks)
        with nc.named_scope("layer1"):
            l1_gts = {c: [None, None] for c in range(nch)}
            ord1 = list(reversed(range(nch)))

            def finish_chunk1(cj):
                emit_chunk(0, cj, chunks[cj], l1_gts[cj])
                if chunks[cj]['b0'] <= CLS_BLK[0] < chunks[cj]['b1']:
                    # blocks 32..48 (class-1 x1~ rows) all written now
                    with nc.named_scope("ag2b"):
                        nc.gpsimd.collective_compute(
                            "AllGather", mybir.AluOpType.bypass,
                            replica_groups=[list(range(C))],
                            ins=[g_own[1][:, :]], outs=[tbl[1][1][:, :]])

            for i in range(nch + 4):
                if i < nch:
                    cj = ord1[i]
                    l1_gts[cj][1] = emit_gather(0, 1, chunks[cj], cj)
                if 3 <= i < nch + 3:
                    ck = ord1[i - 3]
                    l1_gts[ck][0] = emit_gather(0, 0, chunks[ck], ck)
                if i >= 4:
                    finish_chunk1(ord1[i - 4])
        # ---- layer 2 (forward order): the first three k1 gather groups are
        # emitted BEFORE ag2a so the in-order Pool stream has work queued
        # while the class-0 AllGather completes; k0 gathers lead blocks
        # by 1 chunk ----
        gts_map = {c: [None, None] for c in range(nch)}
        with nc.named_scope("layer2"):
            for i in range(3):
                gts_map[i][1] = emit_gather(1, 1, chunks[i], i)
        with nc.named_scope("ag2a"):
            nc.gpsimd.collective_compute(
                "AllGather", mybir.AluOpType.bypass,
                replica_groups=[list(range(C))],
                ins=[g_own[0][:, :]], outs=[tbl[1][0][:, :]])
        with nc.named_scope("layer2"):
            for i in range(3, nch + 4):
                if i < nch:
                    gts_map[i][1] = emit_gather(1, 1, chunks[i], i)
                if i - 3 < nch:
                    gts_map[i - 3][0] = emit_gather(1, 0, chunks[i - 3], i - 3)
                if i >= 4:
                    emit_chunk(1, i - 4, chunks[i - 4], gts_map[i - 4])
    return nc


# ---------------------------------------------------------------------------
# Entry point
# ---------------------------------------------------------------------------

def make_inputs(x, conv_w, conv_b, res_w, res_b, meta):
    npc, npad, B = meta['npc'], meta['npad'], meta['B']
    iota = np.tile(np.arange(P, dtype=np.float32), (P, 1)).astype(bf16)
    warm = np.zeros((128, 8), dtype=np.int16)
    xf = np.asarray(x, dtype=np.float32)
    # xpr: [p, (c*B + j)*128 + f] = x[c*npc + j*128 + p, f]
    xpad = np.zeros((C, npad, D), dtype=np.float32)
    for c in range(C):
        xpad[c, :npc] = xf[c * npc:(c + 1) * npc]
    # [C, B, P, D] -> [P, C, B, D]
    xpr = xpad.reshape(C, B, P, D).transpose(2, 0, 1, 3) \
              .reshape(P, C * B * D).astype(bf16)
    degf_full = np.concatenate([meta['deg'][o] for o in range(C)], axis=1)
    in_maps = []
    for c in range(C):
        xT = np.zeros((P, npad), dtype=bf16)
        xT[:, :npc] = xf[c * npc:(c + 1) * npc].T.astype(bf16)
        xprme = xpad[c].reshape(B, P, D).transpose(1, 0, 2) \
                       .reshape(P, B * D).astype(bf16)
        in_maps.append({
            "xpr": xpr,
            "xprme": xprme,
            "xT": xT,
            "degf": degf_full,
            "w1": np.asarray(conv_w[0], dtype=np.float32),
            "w2": np.asarray(conv_w[1], dtype=np.float32),
            "resw": np.asarray(res_w, dtype=np.float32),
            "convb": np.asarray(conv_b, dtype=np.float32),
            "resb": np.asarray(res_b, dtype=np.float32).reshape(1, D),
            "deg": meta['deg'][c],
            "idx0": meta['idx'][0][c],
            "idx1": meta['idx'][1][c],
            "slot0": meta['slots'][0][c].astype(bf16),
            "slot1": meta['slots'][1][c].astype(bf16),
            "warmidx": warm,
            "iota": iota,
            "ident": np.eye(P, dtype=np.float32),
            "ones": np.ones((1, D), dtype=np.float32).astype(bf16),
        })
    return in_maps


def run(x, edge_index, conv_w, conv_b, res_w, res_b, trace=False, trace_kwargs=None):
    N = x.shape[0]
    meta = prep(edge_index, N)
    nc = build_program(meta)
    nc.compile()
    fix_gather_queues(nc)
    split_sync_waits(nc)
    in_maps = make_inputs(x, conv_w, conv_b, res_w, res_b, meta)
    res = run_bass_kernel_spmd(nc, in_maps, list(range(C)), trace=trace,
                               **(trace_kwargs or {}))
    npc = meta['npc']
    out = np.concatenate([np.asarray(res.results[c]["out"])[:npc]
                          for c in range(C)], axis=0)
    return out.astype(np.float32), res


def kernel(x, edge_index, conv_w, conv_b, res_w, res_b):
    out, _ = run(x, edge_index, conv_w, conv_b, res_w, res_b, trace=False)
    return out
